# revision 17
# baseline (speedup 1.0000x reference)
"""MoE LoRA linear kernel for Trainium2, 8 NeuronCores, data-parallel over tokens.

Reference computation (per token x, D=4096, E=28 experts, rank 8, top-2):
  base   = x @ W^T
  logits = x @ gate_W^T ; top-2 softmax -> per-expert gates g (0 elsewhere)
  h_e    = x @ A_e^T                     (all experts, rank 8)
  out    = base + sum_e g_e*2 * h_e @ B_e^T

Sharding: tokens split 8 ways (1024 tokens/core); weights replicated.

Numerics: everything in bf16 (inputs cast on host, fp32 PSUM accumulate,
bf16 output upcast on host).  Measured end-to-end rel err ~3e-3 vs the
fp32 reference (tolerance 2e-2).

Structure (per core):
  x is staged token-tile-major ([P, TT, KT, P] resident bf16, one DMA per
  token tile).  gate_W is folded into the lora-A operand (cols 224-251),
  so one 256-wide matmul per (tile, kt) yields both h and the gate logits;
  per-tile top-2 softmax chains run on DVE/Act behind the PE.
  Group 0 (output rows 0-2) holds all 32 of its W k-tiles in SBUF: its
  chunk-0 k-loop streams W and interleaves the gating/lora-h work for all
  8 token tiles as hooks; its chunk-1 k-loop reuses the held tiles (W is
  streamed exactly once).  Groups 1..11 stream W per k-tile with 6 PSUM
  banks (3 row-tiles x 2 token chunks), finishing each group with the 4
  lora rank-combine matmuls accumulated into the same PSUM, copy-out
  (fp32->bf16), store.
"""
import sys

for _p in ("/opt/trn_rl_repo", "/root/.axon_site/_ro/trn_rl_repo"):
    if _p not in sys.path:
        sys.path.insert(0, _p)

import numpy as np

import concourse.bass as bass
import concourse.mybir as mybir
import concourse.tile as tile
from concourse import bacc, bass_utils
from concourse.masks import make_identity

F32 = mybir.dt.float32
BF16 = mybir.dt.bfloat16
NP_BF16 = mybir.dt.np(BF16)

N_CORES = 8
B, S, D_IN, D_OUT = 4, 2048, 4096, 4096
N_EXPERTS, RANK, SCALING = 28, 8, 2.0
ER = N_EXPERTS * RANK          # 224
T = B * S // N_CORES           # 1024 tokens per core
P = 128
KT = D_IN // P                 # 32 k-tiles
JT = D_OUT // P                # 32 output row-tiles
TT = T // P                    # 8 token tiles
NCH = 512                      # moving free dim chunk
JG = 3                         # j-tiles per psum group (3x2 chunks = 6 banks)
AT_COLS = 256                  # 224 lora + 28 gate + 4 pad


def build_nc():
    nc = bacc.Bacc("TRN2", target_bir_lowering=False, debug=False)
    # tiles 0-3 packed as 4 k-slabs (each: all 4 tiles x 8 k-tiles) so the
    # base GEMM can start after slab 0; tiles 4-7 whole-tile.
    xg0_d = nc.dram_tensor("xg0", [4, P, 4 * (KT // 4) * P], BF16,
                           kind="ExternalInput").ap()
    xh_d = nc.dram_tensor("xh", [4, P, KT * P], BF16,
                          kind="ExternalInput").ap()
    wt_d = nc.dram_tensor("wt", [D_IN, D_OUT], BF16, kind="ExternalInput").ap()
    at_d = nc.dram_tensor("at", [P, KT * AT_COLS], BF16,
                          kind="ExternalInput").ap()
    ba_d = nc.dram_tensor("ba", [P, D_OUT], BF16, kind="ExternalInput").ap()
    bb_d = nc.dram_tensor("bb", [P, D_OUT], BF16, kind="ExternalInput").ap()
    out_d = nc.dram_tensor("out", [D_OUT, T], BF16, kind="ExternalOutput").ap()

    at_re = at_d.rearrange("p (kt c) -> p kt c", kt=KT)
    xh_re = xh_d.rearrange("t p (kt i) -> t p kt i", kt=KT)
    xg0_re = xg0_d.rearrange("q p (t kq i) -> q p t kq i", t=4, kq=KT // 4)

    groups = [(g * JG, JG) for g in range(JT // JG)]
    if JT % JG:
        groups.append((JT - JT % JG, JT % JG))

    with tile.TileContext(nc) as tc:
        with (
            tc.tile_pool(name="resident", bufs=1) as rp,
            tc.tile_pool(name="wstream", bufs=38) as wp,
            tc.tile_pool(name="bstream", bufs=2) as bp,
            tc.tile_pool(name="outstage", bufs=3) as op_,
            tc.tile_pool(name="smalls", bufs=2) as sp,
            tc.tile_pool(name="gating", bufs=1) as gp,
            tc.tile_pool(name="ph1ps", bufs=2, space="PSUM") as ph1,
            tc.tile_pool(name="psmm", bufs=6, space="PSUM") as psm,
        ):
            ident = rp.tile([P, P], BF16)
            make_identity(nc, ident[:])
            xt_sb = rp.tile([P, TT, KT, P], BF16)
            at_sb = rp.tile([P, KT, AT_COLS], BF16)
            # k-slab 0 of tiles 0-3 first (unblocks the base GEMM), at
            # interleaved early (unblocks lora-h), remaining slabs, then
            # tiles 4-7 whole.
            KQ = KT // 4

            def load_slab(q):
                qs = slice(q * KQ, (q + 1) * KQ)
                nc.sync.dma_start(xt_sb[:, 0:4, qs, :], xg0_re[q])

            load_slab(0)
            nc.sync.dma_start(at_sb[:, 0:KT // 2], at_re[:, 0:KT // 2])
            load_slab(1)
            nc.sync.dma_start(at_sb[:, KT // 2:], at_re[:, KT // 2:])
            load_slab(2)
            load_slab(3)
            for t in range(4, TT):
                nc.sync.dma_start(xt_sb[:, t], xh_re[t - 4])
            hta_sb = rp.tile([P, T], BF16)
            htb_sb = rp.tile([P, T], BF16)
            logits_all = rp.tile([P, TT, N_EXPERTS], F32)
            gsc_all = rp.tile([P, TT, AT_COLS // RANK], F32)

            def gate_chain(t):
                """Top-2 softmax for token tile t (fp32, DVE+Act)."""
                EB = (P, 1, N_EXPERTS)
                sl = slice(t, t + 1)
                m1 = gp.tile([P, 1], F32, name=f"m1_{t}", tag="m1")
                nc.vector.reduce_max(m1[:], logits_all[:, sl],
                                     axis=mybir.AxisListType.X)
                m1b = m1[:, :, None].to_broadcast(EB)
                eq = gp.tile([P, 1, N_EXPERTS], F32, name=f"eq_{t}", tag="eq")
                nc.vector.tensor_tensor(eq[:], logits_all[:, sl], m1b,
                                        mybir.AluOpType.is_equal)
                nc.vector.scalar_tensor_tensor(
                    eq[:], eq[:], -1e30, logits_all[:, sl],
                    mybir.AluOpType.mult, mybir.AluOpType.add)
                m2 = gp.tile([P, 1], F32, name=f"m2_{t}", tag="m2")
                nc.vector.reduce_max(m2[:], eq[:], axis=mybir.AxisListType.X)
                mask2 = gp.tile([P, 1, N_EXPERTS], F32, name=f"mask2_{t}",
                                tag="mask2")
                nc.vector.tensor_tensor(mask2[:], logits_all[:, sl],
                                        m2[:, :, None].to_broadcast(EB),
                                        mybir.AluOpType.is_ge)
                d1 = gp.tile([P, 1, N_EXPERTS], F32, name=f"d1_{t}", tag="d1")
                nc.vector.tensor_tensor(d1[:], logits_all[:, sl], m1b,
                                        mybir.AluOpType.subtract)
                nc.scalar.activation(d1[:], d1[:],
                                     mybir.ActivationFunctionType.Exp)
                d2 = gp.tile([P, 1], F32, name=f"d2_{t}", tag="d2")
                nc.vector.tensor_tensor(d2[:], m2[:], m1[:],
                                        mybir.AluOpType.subtract)
                nc.scalar.activation(d2[:], d2[:],
                                     mybir.ActivationFunctionType.Exp)
                nc.vector.tensor_scalar_add(d2[:], d2[:], 1.0)
                nc.vector.reciprocal(d2[:], d2[:])
                nc.vector.tensor_scalar_mul(d2[:], d2[:], SCALING)
                nc.vector.memset(gsc_all[:, sl, N_EXPERTS:], 0.0)
                nc.vector.tensor_tensor(d1[:], d1[:], mask2[:],
                                        mybir.AluOpType.mult)
                nc.vector.tensor_tensor(gsc_all[:, sl, :N_EXPERTS], d1[:],
                                        d2[:, :, None].to_broadcast(EB),
                                        mybir.AluOpType.mult)

            hps = {}

            def h_a(t):
                """lora-h + gate matmuls for tile t, softmax chain, weight.
                Only the 32 matmuls occupy PE; the rest queues on DVE/Act."""
                ph_ = ph1.tile([P, AT_COLS], F32, name="ph", tag="ph1")
                for kt in range(KT):
                    nc.tensor.matmul(ph_[:], xt_sb[:, t, kt], at_sb[:, kt],
                                     start=(kt == 0), stop=(kt == KT - 1))
                nc.vector.tensor_copy(logits_all[:, t],
                                      ph_[:, ER:ER + N_EXPERTS])
                gate_chain(t)
                hp = sp.tile([P, AT_COLS], BF16, name="hp")
                # gsc is zero for cols >= 224 (experts 28-31), so the gate
                # logit columns of hp are zeroed here.
                nc.vector.tensor_tensor(
                    hp[:].rearrange("p (e r) -> p e r", r=RANK),
                    ph_[:].rearrange("p (e r) -> p e r", r=RANK),
                    gsc_all[:, t, :, None].to_broadcast(
                        (P, AT_COLS // RANK, RANK)),
                    mybir.AluOpType.mult)
                hps[t] = hp

            def h_b(t):
                """Transposes for tile t -> h'T columns (PE, tiny)."""
                ts_ = slice(t * P, (t + 1) * P)
                hp = hps.pop(t)
                for half, dst in ((0, hta_sb), (1, htb_sb)):
                    pt = ph1.tile([P, P], BF16, name="pt", tag="ph1")
                    nc.tensor.transpose(
                        pt[:], hp[:, half * P:(half + 1) * P], ident[:])
                    nc.vector.tensor_copy(dst[:, ts_], pt[:])

            hooks0 = {18: lambda: h_a(0), 20: lambda: h_b(0),
                      21: lambda: h_a(1), 23: lambda: h_b(1),
                      24: lambda: h_a(2), 26: lambda: h_b(2),
                      27: lambda: h_a(3), 29: lambda: h_b(3),
                      30: lambda: h_a(4)}
            hooks1 = {0: lambda: h_b(4), 2: lambda: h_a(5),
                      5: lambda: h_b(5), 8: lambda: h_a(6),
                      11: lambda: h_b(6), 14: lambda: h_a(7),
                      17: lambda: h_b(7)}

            b_tiles = {}

            def load_b(gi):
                j0, nj = groups[gi]
                js = slice(j0 * P, (j0 + nj) * P)
                ba_t = bp.tile([P, JG * P], BF16, name="ba_t")
                nc.scalar.dma_start(ba_t[:, :nj * P], ba_d[:, js])
                bb_t = bp.tile([P, JG * P], BF16, name="bb_t")
                nc.scalar.dma_start(bb_t[:, :nj * P], bb_d[:, js])
                b_tiles[gi] = (ba_t, bb_t)

            def combine_and_store(gi, psums):
                j0, nj = groups[gi]
                ba_t, bb_t = b_tiles.pop(gi)
                for j in range(nj):
                    for c in range(2):
                        cs = slice(c * NCH, (c + 1) * NCH)
                        nc.tensor.matmul(
                            psums[j, c], ba_t[:, j * P:(j + 1) * P],
                            hta_sb[:, cs], start=False, stop=False)
                        nc.tensor.matmul(
                            psums[j, c], bb_t[:, j * P:(j + 1) * P],
                            htb_sb[:, cs], start=False, stop=True)
                        ot = op_.tile([P, NCH], BF16, name="ot")
                        nc.vector.tensor_copy(ot[:], psums[j, c])
                        nc.sync.dma_start(
                            out_d[(j0 + j) * P:(j0 + j + 1) * P, cs], ot[:])

            load_b(0)
            # ---- group 0: rows 0-2, holds its 32 W k-tiles ----
            psums0 = {
                (j, c): psm.tile([P, NCH], F32, name=f"pm_{j}_{c}", tag="pm")
                for j in range(JG) for c in range(2)
            }
            g0_w = []
            for kt in range(KT):
                w_t = wp.tile([P, JG * P], BF16, name="w_t")
                nc.scalar.dma_start(w_t[:], wt_d[kt * P:(kt + 1) * P, :JG * P])
                g0_w.append(w_t)
                if kt == 2:
                    load_b(1)
                for j in range(JG):
                    nc.tensor.matmul(
                        psums0[j, 0], w_t[:, j * P:(j + 1) * P],
                        xt_sb[:, 0:4, kt, :], start=(kt == 0), stop=False)
                if kt in hooks0:
                    hooks0[kt]()
            for kt in range(KT):
                w_t = g0_w[kt]
                for j in range(JG):
                    nc.tensor.matmul(
                        psums0[j, 1], w_t[:, j * P:(j + 1) * P],
                        xt_sb[:, 4:8, kt, :], start=(kt == 0), stop=False)
                if kt in hooks1:
                    hooks1[kt]()
            combine_and_store(0, psums0)

            # ---- groups 1..: stream W per k-tile ----
            for gi in range(1, len(groups)):
                j0, nj = groups[gi]
                js = slice(j0 * P, (j0 + nj) * P)
                psums = {
                    (j, c): psm.tile([P, NCH], F32, name=f"pm_{j}_{c}",
                                     tag="pm")
                    for j in range(nj) for c in range(2)
                }
                for kt in range(KT):
                    w_t = wp.tile([P, JG * P], BF16, name="w_t")
                    nc.scalar.dma_start(
                        w_t[:, :nj * P], wt_d[kt * P:(kt + 1) * P, js])
                    if kt == 2 and gi + 1 < len(groups):
                        load_b(gi + 1)
                    for j in range(nj):
                        lhs = w_t[:, j * P:(j + 1) * P]
                        for c in range(2):
                            nc.tensor.matmul(
                                psums[j, c], lhs,
                                xt_sb[:, 4 * c:4 * (c + 1), kt, :],
                                start=(kt == 0), stop=False)
                combine_and_store(gi, psums)
    nc.compile()
    return nc


_NC_CACHE = None
_LAST_IN_MAPS = None


def _get_nc():
    global _NC_CACHE
    if _NC_CACHE is None:
        _NC_CACHE = build_nc()
    return _NC_CACHE


def kernel(x, base_W, gate_W, lora_A, lora_B):
    x = np.asarray(x, dtype=np.float32)
    base_W = np.asarray(base_W, dtype=np.float32)
    gate_W = np.asarray(gate_W, dtype=np.float32)
    lora_A = np.asarray(lora_A, dtype=np.float32)
    lora_B = np.asarray(lora_B, dtype=np.float32)

    xf = x.reshape(B * S, D_IN)
    wt_np = np.ascontiguousarray(base_W.T).astype(NP_BF16)       # [D_in, D_out]
    # lora_A [E, R, D_in] -> at cols 0-223; gate_W -> cols 224-251; pad 252-255
    a_flat = lora_A.reshape(ER, D_IN)
    at_np = np.zeros((D_IN, AT_COLS), dtype=np.float32)
    at_np[:, :ER] = a_flat.T
    at_np[:, ER:ER + N_EXPERTS] = gate_W.T
    at_np = np.ascontiguousarray(
        at_np.reshape(KT, P, AT_COLS).transpose(1, 0, 2).reshape(
            P, KT * AT_COLS)).astype(NP_BF16)
    # lora_B [E, D_out, R] -> b_flat [(e r), D_out] -> halves split at er=128
    b_flat = np.ascontiguousarray(
        lora_B.transpose(0, 2, 1).reshape(ER, D_OUT))
    ba_np = np.zeros((P, D_OUT), dtype=np.float32)
    bb_np = np.zeros((P, D_OUT), dtype=np.float32)
    ba_np[:] = b_flat[:P]
    bb_np[:ER - P] = b_flat[P:]
    ba_np = ba_np.astype(NP_BF16)
    bb_np = bb_np.astype(NP_BF16)

    in_maps = []
    for c in range(N_CORES):
        xs = xf[c * T:(c + 1) * T].astype(NP_BF16)               # [T, D_in]
        xr = xs.reshape(TT, P, KT, P)          # [t, i(tok), kt, p(d)]
        # tiles 0-3 as k-slabs: xg0[q, p, t, kt', i] with kt = 8q + kt'
        xg0_np = np.ascontiguousarray(
            xr[:4].reshape(4, P, 4, KT // 4, P).transpose(2, 4, 0, 3, 1)
            .reshape(4, P, 4 * (KT // 4) * P))
        # tiles 4-7 whole: xh[t, p, kt*P+i] = xs[(t+4)*P+i, kt*P+p]
        xh_np = np.ascontiguousarray(
            xr[4:].transpose(0, 3, 2, 1).reshape(4, P, KT * P))
        in_maps.append({
            "xg0": xg0_np,
            "xh": xh_np,
            "wt": wt_np,
            "at": at_np,
            "ba": ba_np,
            "bb": bb_np,
        })

    global _LAST_IN_MAPS
    _LAST_IN_MAPS = in_maps
    nc = _get_nc()
    res = bass_utils.run_bass_kernel_spmd(nc, in_maps,
                                          core_ids=list(range(N_CORES)))
    out = np.empty((B * S, D_OUT), dtype=np.float32)
    for c in range(N_CORES):
        out[c * T:(c + 1) * T] = res.results[c]["out"].astype(np.float32).T
    return out.reshape(B, S, D_OUT)


# revision 19
# speedup vs baseline: 1.3617x; 1.3617x over previous
"""MoE LoRA linear kernel for Trainium2, 8 NeuronCores, data-parallel over tokens.

Reference computation (per token x, D=4096, E=28 experts, rank 8, top-2):
  base   = x @ W^T
  logits = x @ gate_W^T ; top-2 softmax -> per-expert gates g (0 elsewhere)
  h_e    = x @ A_e^T                     (all experts, rank 8)
  out    = base + sum_e g_e*2 * h_e @ B_e^T

Sharding: tokens split 8 ways (1024 tokens/core); weights replicated.

Numerics: everything in bf16 (inputs cast on host, fp32 PSUM accumulate,
bf16 output upcast on host).  Measured end-to-end rel err ~3e-3 vs the
fp32 reference (tolerance 2e-2).

Structure (per core):
  x is staged token-tile-major ([P, TT, KT, P] resident bf16, one DMA per
  token tile).  gate_W is folded into the lora-A operand (cols 224-251),
  so one 256-wide matmul per (tile, kt) yields both h and the gate logits;
  per-tile top-2 softmax chains run on DVE/Act behind the PE.
  Group 0 (output rows 0-2) holds all 32 of its W k-tiles in SBUF: its
  chunk-0 k-loop streams W and interleaves the gating/lora-h work for all
  8 token tiles as hooks; its chunk-1 k-loop reuses the held tiles (W is
  streamed exactly once).  Groups 1..11 stream W per k-tile with 6 PSUM
  banks (3 row-tiles x 2 token chunks), finishing each group with the 4
  lora rank-combine matmuls accumulated into the same PSUM, copy-out
  (fp32->bf16), store.
"""
import sys

for _p in ("/opt/trn_rl_repo", "/root/.axon_site/_ro/trn_rl_repo"):
    if _p not in sys.path:
        sys.path.insert(0, _p)

import numpy as np

import concourse.bass as bass
import concourse.mybir as mybir
import concourse.tile as tile
from concourse import bacc, bass_utils
from concourse.masks import make_identity

F32 = mybir.dt.float32
BF16 = mybir.dt.bfloat16
NP_BF16 = mybir.dt.np(BF16)

N_CORES = 8
B, S, D_IN, D_OUT = 4, 2048, 4096, 4096
N_EXPERTS, RANK, SCALING = 28, 8, 2.0
ER = N_EXPERTS * RANK          # 224
T = B * S // N_CORES           # 1024 tokens per core
P = 128
KT = D_IN // P                 # 32 k-tiles
JT = D_OUT // P                # 32 output row-tiles
TT = T // P                    # 8 token tiles
NCH = 512                      # moving free dim chunk
JG = 3                         # j-tiles per psum group (3x2 chunks = 6 banks)
AT_COLS = 256                  # 224 lora + 28 gate + 4 pad


def build_nc():
    nc = bacc.Bacc("TRN2", target_bir_lowering=False, debug=False)
    # tiles 0-3 packed as 4 k-slabs (each: all 4 tiles x 8 k-tiles) so the
    # base GEMM can start after slab 0; tiles 4-7 whole-tile.
    xg0_d = nc.dram_tensor("xg0", [4, P, 4 * (KT // 4) * P], BF16,
                           kind="ExternalInput").ap()
    xh_d = nc.dram_tensor("xh", [4, P, KT * P], BF16,
                          kind="ExternalInput").ap()
    wt_d = nc.dram_tensor("wt", [D_IN, D_OUT], BF16, kind="ExternalInput").ap()
    at_d = nc.dram_tensor("at", [P, KT * AT_COLS], BF16,
                          kind="ExternalInput").ap()
    ba_d = nc.dram_tensor("ba", [P, D_OUT], BF16, kind="ExternalInput").ap()
    bb_d = nc.dram_tensor("bb", [P, D_OUT], BF16, kind="ExternalInput").ap()
    out_d = nc.dram_tensor("out", [D_OUT, T], BF16, kind="ExternalOutput").ap()

    at_re = at_d.rearrange("p (kt c) -> p kt c", kt=KT)
    xh_re = xh_d.rearrange("t p (kt i) -> t p kt i", kt=KT)
    xg0_re = xg0_d.rearrange("q p (t kq i) -> q p t kq i", t=4, kq=KT // 4)

    groups = [(g * JG, JG) for g in range(JT // JG)]
    if JT % JG:
        groups.append((JT - JT % JG, JT % JG))

    with tile.TileContext(nc) as tc:
        with (
            tc.tile_pool(name="resident", bufs=1) as rp,
            tc.tile_pool(name="wstream", bufs=38) as wp,
            tc.tile_pool(name="bstream", bufs=2) as bp,
            tc.tile_pool(name="outstage", bufs=3) as op_,
            tc.tile_pool(name="smalls", bufs=2) as sp,
            tc.tile_pool(name="gating", bufs=1) as gp,
            tc.tile_pool(name="ph1ps", bufs=2, space="PSUM") as ph1,
            tc.tile_pool(name="psmm", bufs=6, space="PSUM") as psm,
        ):
            ident = rp.tile([P, P], BF16)
            make_identity(nc, ident[:])
            xt_sb = rp.tile([P, TT, KT, P], BF16)
            at_sb = rp.tile([P, KT, AT_COLS], BF16)
            # k-slab 0 of tiles 0-3 first (unblocks the base GEMM), at
            # interleaved early (unblocks lora-h), remaining slabs, then
            # tiles 4-7 whole.
            KQ = KT // 4

            def load_slab(q):
                qs = slice(q * KQ, (q + 1) * KQ)
                nc.sync.dma_start(xt_sb[:, 0:4, qs, :], xg0_re[q])

            load_slab(0)
            nc.sync.dma_start(at_sb[:, 0:KT // 2], at_re[:, 0:KT // 2])
            load_slab(1)
            nc.sync.dma_start(at_sb[:, KT // 2:], at_re[:, KT // 2:])
            load_slab(2)
            load_slab(3)
            for t in range(4, TT):
                nc.sync.dma_start(xt_sb[:, t], xh_re[t - 4])
            hta_sb = rp.tile([P, T], BF16)
            htb_sb = rp.tile([P, T], BF16)
            logits_all = rp.tile([P, TT, N_EXPERTS], F32)
            gsc_all = rp.tile([P, TT, AT_COLS // RANK], F32)

            def gate_chain(t):
                """Top-2 softmax for token tile t (fp32, DVE+Act)."""
                EB = (P, 1, N_EXPERTS)
                sl = slice(t, t + 1)
                m1 = gp.tile([P, 1], F32, name=f"m1_{t}", tag="m1")
                nc.vector.reduce_max(m1[:], logits_all[:, sl],
                                     axis=mybir.AxisListType.X)
                m1b = m1[:, :, None].to_broadcast(EB)
                eq = gp.tile([P, 1, N_EXPERTS], F32, name=f"eq_{t}", tag="eq")
                nc.vector.tensor_tensor(eq[:], logits_all[:, sl], m1b,
                                        mybir.AluOpType.is_equal)
                nc.vector.scalar_tensor_tensor(
                    eq[:], eq[:], -1e30, logits_all[:, sl],
                    mybir.AluOpType.mult, mybir.AluOpType.add)
                m2 = gp.tile([P, 1], F32, name=f"m2_{t}", tag="m2")
                nc.vector.reduce_max(m2[:], eq[:], axis=mybir.AxisListType.X)
                mask2 = gp.tile([P, 1, N_EXPERTS], F32, name=f"mask2_{t}",
                                tag="mask2")
                nc.vector.tensor_tensor(mask2[:], logits_all[:, sl],
                                        m2[:, :, None].to_broadcast(EB),
                                        mybir.AluOpType.is_ge)
                d1 = gp.tile([P, 1, N_EXPERTS], F32, name=f"d1_{t}", tag="d1")
                nc.vector.tensor_tensor(d1[:], logits_all[:, sl], m1b,
                                        mybir.AluOpType.subtract)
                nc.scalar.activation(d1[:], d1[:],
                                     mybir.ActivationFunctionType.Exp)
                d2 = gp.tile([P, 1], F32, name=f"d2_{t}", tag="d2")
                nc.vector.tensor_tensor(d2[:], m2[:], m1[:],
                                        mybir.AluOpType.subtract)
                nc.scalar.activation(d2[:], d2[:],
                                     mybir.ActivationFunctionType.Exp)
                nc.vector.tensor_scalar_add(d2[:], d2[:], 1.0)
                nc.vector.reciprocal(d2[:], d2[:])
                nc.vector.tensor_scalar_mul(d2[:], d2[:], SCALING)
                nc.vector.memset(gsc_all[:, sl, N_EXPERTS:], 0.0)
                nc.vector.tensor_tensor(d1[:], d1[:], mask2[:],
                                        mybir.AluOpType.mult)
                nc.vector.tensor_tensor(gsc_all[:, sl, :N_EXPERTS], d1[:],
                                        d2[:, :, None].to_broadcast(EB),
                                        mybir.AluOpType.mult)

            hps = {}

            def h_a(t):
                """lora-h + gate matmuls for tile t, softmax chain, weight.
                Only the 32 matmuls occupy PE; the rest queues on DVE/Act."""
                ph_ = ph1.tile([P, AT_COLS], F32, name="ph", tag="ph1")
                for kt in range(KT):
                    nc.tensor.matmul(ph_[:], xt_sb[:, t, kt], at_sb[:, kt],
                                     start=(kt == 0), stop=(kt == KT - 1))
                nc.vector.tensor_copy(logits_all[:, t],
                                      ph_[:, ER:ER + N_EXPERTS])
                gate_chain(t)
                hp = sp.tile([P, AT_COLS], BF16, name="hp")
                # gsc is zero for cols >= 224 (experts 28-31), so the gate
                # logit columns of hp are zeroed here.
                nc.vector.tensor_tensor(
                    hp[:].rearrange("p (e r) -> p e r", r=RANK),
                    ph_[:].rearrange("p (e r) -> p e r", r=RANK),
                    gsc_all[:, t, :, None].to_broadcast(
                        (P, AT_COLS // RANK, RANK)),
                    mybir.AluOpType.mult)
                hps[t] = hp

            def h_b(t):
                """Transposes for tile t -> h'T columns (PE, tiny)."""
                ts_ = slice(t * P, (t + 1) * P)
                hp = hps.pop(t)
                for half, dst in ((0, hta_sb), (1, htb_sb)):
                    pt = ph1.tile([P, P], BF16, name="pt", tag="ph1")
                    nc.tensor.transpose(
                        pt[:], hp[:, half * P:(half + 1) * P], ident[:])
                    nc.vector.tensor_copy(dst[:, ts_], pt[:])

            hooks0 = {18: lambda: h_a(0), 20: lambda: h_b(0),
                      21: lambda: h_a(1), 23: lambda: h_b(1),
                      24: lambda: h_a(2), 26: lambda: h_b(2),
                      27: lambda: h_a(3), 29: lambda: h_b(3),
                      30: lambda: h_a(4)}
            hooks1 = {0: lambda: h_b(4), 2: lambda: h_a(5),
                      5: lambda: h_b(5), 8: lambda: h_a(6),
                      11: lambda: h_b(6), 14: lambda: h_a(7),
                      17: lambda: h_b(7)}

            b_tiles = {}

            def load_b(gi):
                j0, nj = groups[gi]
                js = slice(j0 * P, (j0 + nj) * P)
                ba_t = bp.tile([P, JG * P], BF16, name="ba_t")
                nc.scalar.dma_start(ba_t[:, :nj * P], ba_d[:, js])
                bb_t = bp.tile([P, JG * P], BF16, name="bb_t")
                nc.scalar.dma_start(bb_t[:, :nj * P], bb_d[:, js])
                b_tiles[gi] = (ba_t, bb_t)

            def combine_and_store(gi, psums):
                j0, nj = groups[gi]
                ba_t, bb_t = b_tiles.pop(gi)
                for j in range(nj):
                    for c in range(2):
                        cs = slice(c * NCH, (c + 1) * NCH)
                        nc.tensor.matmul(
                            psums[j, c], ba_t[:, j * P:(j + 1) * P],
                            hta_sb[:, cs], start=False, stop=False)
                        nc.tensor.matmul(
                            psums[j, c], bb_t[:, j * P:(j + 1) * P],
                            htb_sb[:, cs], start=False, stop=True)
                        ot = op_.tile([P, NCH], BF16, name="ot")
                        nc.vector.tensor_copy(ot[:], psums[j, c])
                        nc.sync.dma_start(
                            out_d[(j0 + j) * P:(j0 + j + 1) * P, cs], ot[:])

            # ---- group 0: rows 0-2, holds its 32 W k-tiles ----
            # b loads are emitted at kt 1/3 so w0/w1 lead the Act FIFO and
            # the first base matmul is not delayed behind them.
            psums0 = {
                (j, c): psm.tile([P, NCH], F32, name=f"pm_{j}_{c}", tag="pm")
                for j in range(JG) for c in range(2)
            }
            g0_w = []
            for kt in range(KT):
                w_t = wp.tile([P, JG * P], BF16, name="w_t")
                nc.scalar.dma_start(w_t[:], wt_d[kt * P:(kt + 1) * P, :JG * P])
                g0_w.append(w_t)
                if kt == 1:
                    load_b(0)
                if kt == 3:
                    load_b(1)
                for j in range(JG):
                    nc.tensor.matmul(
                        psums0[j, 0], w_t[:, j * P:(j + 1) * P],
                        xt_sb[:, 0:4, kt, :], start=(kt == 0), stop=False)
                if kt in hooks0:
                    hooks0[kt]()
            for kt in range(KT):
                w_t = g0_w[kt]
                for j in range(JG):
                    nc.tensor.matmul(
                        psums0[j, 1], w_t[:, j * P:(j + 1) * P],
                        xt_sb[:, 4:8, kt, :], start=(kt == 0), stop=False)
                if kt in hooks1:
                    hooks1[kt]()
            combine_and_store(0, psums0)

            # ---- groups 1..: stream W per k-tile ----
            for gi in range(1, len(groups)):
                j0, nj = groups[gi]
                js = slice(j0 * P, (j0 + nj) * P)
                psums = {
                    (j, c): psm.tile([P, NCH], F32, name=f"pm_{j}_{c}",
                                     tag="pm")
                    for j in range(nj) for c in range(2)
                }
                for kt in range(KT):
                    w_t = wp.tile([P, JG * P], BF16, name="w_t")
                    nc.scalar.dma_start(
                        w_t[:, :nj * P], wt_d[kt * P:(kt + 1) * P, js])
                    if kt == 2 and gi + 1 < len(groups):
                        load_b(gi + 1)
                    for j in range(nj):
                        lhs = w_t[:, j * P:(j + 1) * P]
                        for c in range(2):
                            nc.tensor.matmul(
                                psums[j, c], lhs,
                                xt_sb[:, 4 * c:4 * (c + 1), kt, :],
                                start=(kt == 0), stop=False)
                combine_and_store(gi, psums)
    nc.compile()
    return nc


_NC_CACHE = None
_LAST_IN_MAPS = None


def _get_nc():
    global _NC_CACHE
    if _NC_CACHE is None:
        _NC_CACHE = build_nc()
    return _NC_CACHE


def kernel(x, base_W, gate_W, lora_A, lora_B):
    x = np.asarray(x, dtype=np.float32)
    base_W = np.asarray(base_W, dtype=np.float32)
    gate_W = np.asarray(gate_W, dtype=np.float32)
    lora_A = np.asarray(lora_A, dtype=np.float32)
    lora_B = np.asarray(lora_B, dtype=np.float32)

    xf = x.reshape(B * S, D_IN)
    wt_np = np.ascontiguousarray(base_W.T).astype(NP_BF16)       # [D_in, D_out]
    # lora_A [E, R, D_in] -> at cols 0-223; gate_W -> cols 224-251; pad 252-255
    a_flat = lora_A.reshape(ER, D_IN)
    at_np = np.zeros((D_IN, AT_COLS), dtype=np.float32)
    at_np[:, :ER] = a_flat.T
    at_np[:, ER:ER + N_EXPERTS] = gate_W.T
    at_np = np.ascontiguousarray(
        at_np.reshape(KT, P, AT_COLS).transpose(1, 0, 2).reshape(
            P, KT * AT_COLS)).astype(NP_BF16)
    # lora_B [E, D_out, R] -> b_flat [(e r), D_out] -> halves split at er=128
    b_flat = np.ascontiguousarray(
        lora_B.transpose(0, 2, 1).reshape(ER, D_OUT))
    ba_np = np.zeros((P, D_OUT), dtype=np.float32)
    bb_np = np.zeros((P, D_OUT), dtype=np.float32)
    ba_np[:] = b_flat[:P]
    bb_np[:ER - P] = b_flat[P:]
    ba_np = ba_np.astype(NP_BF16)
    bb_np = bb_np.astype(NP_BF16)

    in_maps = []
    for c in range(N_CORES):
        xs = xf[c * T:(c + 1) * T].astype(NP_BF16)               # [T, D_in]
        xr = xs.reshape(TT, P, KT, P)          # [t, i(tok), kt, p(d)]
        # tiles 0-3 as k-slabs: xg0[q, p, t, kt', i] with kt = 8q + kt'
        xg0_np = np.ascontiguousarray(
            xr[:4].reshape(4, P, 4, KT // 4, P).transpose(2, 4, 0, 3, 1)
            .reshape(4, P, 4 * (KT // 4) * P))
        # tiles 4-7 whole: xh[t, p, kt*P+i] = xs[(t+4)*P+i, kt*P+p]
        xh_np = np.ascontiguousarray(
            xr[4:].transpose(0, 3, 2, 1).reshape(4, P, KT * P))
        in_maps.append({
            "xg0": xg0_np,
            "xh": xh_np,
            "wt": wt_np,
            "at": at_np,
            "ba": ba_np,
            "bb": bb_np,
        })

    global _LAST_IN_MAPS
    _LAST_IN_MAPS = in_maps
    nc = _get_nc()
    res = bass_utils.run_bass_kernel_spmd(nc, in_maps,
                                          core_ids=list(range(N_CORES)))
    out = np.empty((B * S, D_OUT), dtype=np.float32)
    for c in range(N_CORES):
        out[c * T:(c + 1) * T] = res.results[c]["out"].astype(np.float32).T
    return out.reshape(B, S, D_OUT)
